# revision 8
# baseline (speedup 1.0000x reference)
"""Discounted cumsum (y[b,h,t,d] = x[b,h,t,d] + gamma[h] * y[b,h,t-1,d]) on 8 trn2 cores.

Pure data parallelism: 64 (b,h) pairs, 8 per core. Two per-pair execution paths
split the work across engines (the DVE scan instruction runs at a fixed ~2
cycles/elem, so the Vector engine alone would cap at ~69 us/core):

- NV pairs on the Vector engine: layout [d=128 partitions, s=4096 free]; the
  whole recurrence is ONE tensor_tensor_scan (state = gamma*state + x, fp32
  internal state). gamma stays EXACT fp32 via a stride-0 broadcast AP - a
  rounded gamma would be raised to the power t by the recurrence, amplifying
  its error by ~t, while independently-rounded coefficient TABLES (PE path)
  and fp16 data are fine at the 2e-2 gate.

- NP pairs on the Tensor engine (blocked parallel scan, single fp16, all-
  "small-scheme"): layout [t-within-block=128 partitions, 32 blocks x 128 d
  free]. Per 4-block group one K=1 inject matmul (gamma^(t+1) x carry) and one
  K=128 scan matmul (A[s,t]=gamma^(t-s)) accumulate in PSUM; block sums come
  from u=gamma^(127-s) matmuls, block carries from one small gamma^128-power
  matmul. PSUM->SBUF copy-outs run on the Scalar engine, the tiny block-sum
  row copies on Scalar/GpSimd, so Vector is untouched by the PE path.

I/O is fp16 both ways (absmax error ~1e-2 against an absolute budget of 0.2),
all transposes are host-side so every DMA is contiguous 8KB lines.
"""

import numpy as np

B, H, S, D = 4, 16, 4096, 128
T = 128          # block length (PE path matmul contraction dim)
KB = S // T      # 32 blocks
NG = 4           # blocks per matmul group (4*128 = 512 moving columns)
G = KB // NG     # 8 groups per pair
NCORES = 8
PAIRS = (B * H) // NCORES  # 8 pair-slots per core
NV = 5           # pairs on the Vector scan path
NP = PAIRS - NV  # pairs on the Tensor (PE) path

_nc_cache = {}


def _build_program():
    if "nc" in _nc_cache:
        return _nc_cache["nc"]

    import concourse.bass as bass
    import concourse.mybir as mybir
    from concourse.tile import TileContext

    f32 = mybir.dt.float32
    fp16 = mybir.dt.float16

    nc = bass.Bass(trn_type="TRN2")

    xv_d = nc.declare_dram_parameter("xv", [NV, D, S], fp16, isOutput=False)
    g_d = nc.declare_dram_parameter("g", [D, NV], f32, isOutput=False)
    yv_d = nc.declare_dram_parameter("yv", [NV, D, S], fp16, isOutput=True)

    xp_d = nc.declare_dram_parameter("xp", [NP, T, KB * D], fp16, isOutput=False)
    A_d = nc.declare_dram_parameter("A_all", [T, NP * T], fp16, isOutput=False)
    u_d = nc.declare_dram_parameter("u_all", [T, NP], fp16, isOutput=False)
    gv_d = nc.declare_dram_parameter("gv_all", [1, NP * T], fp16, isOutput=False)
    GT_d = nc.declare_dram_parameter("GT_all", [KB, NP * KB], fp16, isOutput=False)
    yp_d = nc.declare_dram_parameter("yp", [NP, T, KB * D], fp16, isOutput=True)

    mult, add = mybir.AluOpType.mult, mybir.AluOpType.add

    with TileContext(nc) as tc:
        with (
            tc.tile_pool(name="const", bufs=1) as cpool,
            tc.tile_pool(name="xvin", bufs=2) as xvpool,
            tc.tile_pool(name="yvout", bufs=2) as yvpool,
            tc.tile_pool(name="xpin", bufs=2) as xppool,
            tc.tile_pool(name="ypout", bufs=2) as yppool,
            tc.tile_pool(name="rfl", bufs=2) as rfpool,
            tc.tile_pool(name="r32", bufs=2) as r32pool,
            tc.tile_pool(name="c32", bufs=2) as c32pool,
            tc.tile_pool(name="cfl", bufs=2) as cfpool,
            tc.tile_pool(name="grp_ps", bufs=4, space="PSUM") as gp_pool,
            tc.tile_pool(name="sum_ps", bufs=2, space="PSUM") as sp_pool,
            tc.tile_pool(name="c_ps", bufs=1, space="PSUM") as cp_pool,
        ):
            gc = cpool.tile([D, NV], f32, tag="gc")
            nc.gpsimd.dma_start(out=gc[:], in_=g_d[:])
            Ac = cpool.tile([T, NP * T], fp16, tag="Ac")
            uc = cpool.tile([T, NP], fp16, tag="uc")
            gvc = cpool.tile([1, NP * T], fp16, tag="gvc")
            GTc = cpool.tile([KB, NP * KB], fp16, tag="GTc")
            nc.gpsimd.dma_start(out=Ac[:], in_=A_d[:])
            nc.gpsimd.dma_start(out=uc[:], in_=u_d[:])
            nc.gpsimd.dma_start(out=gvc[:], in_=gv_d[:])
            nc.gpsimd.dma_start(out=GTc[:], in_=GT_d[:])

            def vector_pair(v):
                X = xvpool.tile([D, S], fp16, tag="Xv")
                nc.sync.dma_start(out=X[:], in_=xv_d[v])
                Y = yvpool.tile([D, S], fp16, tag="Yv")
                nc.vector.tensor_tensor_scan(
                    out=Y[:],
                    data0=gc[:, v : v + 1].broadcast_to([D, S]),
                    data1=X[:],
                    initial=0.0,
                    op0=mult,
                    op1=add,
                )
                nc.scalar.dma_start(out=yv_d[v], in_=Y[:])

            def pe_pair(q):
                X = xppool.tile([T, KB * D], fp16, tag="Xp")
                nc.sync.dma_start(out=X[:], in_=xp_d[q])

                # block sums r_k = sum_s gamma^(127-s) x_s  -> [1, (k d)]
                Rflat = rfpool.tile([1, KB * D], fp16, tag="Rflat")
                for g in range(G):
                    sl = slice(g * NG * D, (g + 1) * NG * D)
                    rp = sp_pool.tile([1, NG * D], f32, tag="rp")
                    nc.tensor.matmul(
                        rp[:], lhsT=uc[:, q : q + 1], rhs=X[:, sl],
                        start=True, stop=True,
                    )
                    nc.scalar.copy(out=Rflat[:, sl], in_=rp[:])
                # scatter [1,(k d)] -> [KB part, d]
                R32 = r32pool.tile([KB, D], fp16, tag="R32")
                nc.sync.dma_start(out=R32[:], in_=Rflat[:])

                # carries C[k] = sum_{j<k} gamma^(128(k-1-j)) r_j
                cp = cp_pool.tile([KB, D], f32, tag="cp")
                nc.tensor.matmul(
                    cp[:], lhsT=GTc[:, q * KB : (q + 1) * KB], rhs=R32[:],
                    start=True, stop=True,
                )
                C32 = c32pool.tile([KB, D], fp16, tag="C32")
                nc.scalar.copy(out=C32[:], in_=cp[:])
                cfh = cfpool.tile([1, KB * D], fp16, tag="cfh")
                nc.sync.dma_start(out=cfh[:], in_=C32[:])

                # per group: inject gamma^(t+1) C_k, then within-block scan
                Ys = yppool.tile([T, KB * D], fp16, tag="Ys")
                Aq = Ac[:, q * T : (q + 1) * T]
                gvq = gvc[:, q * T : (q + 1) * T]
                for g in range(G):
                    sl = slice(g * NG * D, (g + 1) * NG * D)
                    grp = gp_pool.tile([T, NG * D], f32, tag="grp")
                    nc.tensor.matmul(
                        grp[:], lhsT=gvq, rhs=cfh[:, sl],
                        start=True, stop=False,
                    )
                    nc.tensor.matmul(
                        grp[:], lhsT=Aq, rhs=X[:, sl],
                        start=False, stop=True,
                    )
                    nc.scalar.copy(out=Ys[:, sl], in_=grp[:])

                nc.scalar.dma_start(out=yp_d[q], in_=Ys[:])

            # Interleave so every engine starts early: V P V P V P V V
            order = []
            vi = iter(range(NV))
            qi = iter(range(NP))
            for p in range(PAIRS):
                if p % 2 == 0 or p >= 2 * NP:
                    order.append(("v", next(vi)))
                else:
                    order.append(("q", next(qi)))
            for kind, idx in order:
                if kind == "v":
                    vector_pair(idx)
                else:
                    pe_pair(idx)

    # Walrus allows 1 sync wait on engine instructions / 2 on DMAs; move
    # excess waits onto InstEventSemaphore carriers.
    import bass_rust

    bass_rust.generate_event_semaphores(nc)

    _nc_cache["nc"] = nc
    return nc


def _pe_constants(g):
    """fp16 coefficient tables from float64 gamma powers."""
    pw = np.power(g, np.arange(S, dtype=np.float64))
    t_idx = np.arange(T)
    t_minus_s = t_idx[None, :] - t_idx[:, None]
    A = np.where(t_minus_s >= 0, pw[np.clip(t_minus_s, 0, None)], 0.0)
    u = pw[127 - t_idx]
    gv = pw[t_idx + 1]
    pw128 = np.power(pw[T], np.arange(KB, dtype=np.float64))
    k_minus_j = np.arange(KB)[None, :] - 1 - np.arange(KB)[:, None]
    GT = np.where(k_minus_j >= 0, pw128[np.clip(k_minus_j, 0, None)], 0.0)
    return A, u, gv, GT


def _make_in_maps(tensor, gamma):
    x = np.asarray(tensor, dtype=np.float32).reshape(B * H, S, D)
    gam = np.asarray(gamma, dtype=np.float32).reshape(H)

    in_maps = []
    for c in range(NCORES):
        pids = [c * PAIRS + p for p in range(PAIRS)]
        # vector-path pairs: [D, S] fp16, scan axis last
        xv = np.empty((NV, D, S), np.float16)
        gcol = np.empty((D, NV), np.float32)
        for v in range(NV):
            pid = pids[v]
            xv[v] = x[pid].T.astype(np.float16)
            gcol[:, v] = gam[pid % H]
        # PE-path pairs: scan layout [s-within-block, (block, d)] fp16
        xp = np.empty((NP, T, KB * D), np.float16)
        A_all = np.empty((T, NP * T), np.float16)
        u_all = np.empty((T, NP), np.float16)
        gv_all = np.empty((1, NP * T), np.float16)
        GT_all = np.empty((KB, NP * KB), np.float16)
        for q in range(NP):
            pid = pids[NV + q]
            xp[q] = (
                x[pid].reshape(KB, T, D).transpose(1, 0, 2).reshape(T, KB * D)
                .astype(np.float16)
            )
            A, u, gv, GT = _pe_constants(float(gam[pid % H]))
            A_all[:, q * T : (q + 1) * T] = A.astype(np.float16)
            u_all[:, q] = u.astype(np.float16)
            gv_all[0, q * T : (q + 1) * T] = gv.astype(np.float16)
            GT_all[:, q * KB : (q + 1) * KB] = GT.astype(np.float16)
        in_maps.append(
            {
                "xv": xv,
                "g": gcol,
                "xp": xp,
                "A_all": A_all,
                "u_all": u_all,
                "gv_all": gv_all,
                "GT_all": GT_all,
            }
        )
    return in_maps


def kernel(tensor, gamma):
    from concourse.bass_utils import run_bass_kernel_spmd

    in_maps = _make_in_maps(tensor, gamma)
    nc = _build_program()
    res = run_bass_kernel_spmd(nc, in_maps, list(range(NCORES))).results
    y = np.empty((B * H, S, D), np.float32)
    for c in range(NCORES):
        yv = np.asarray(res[c]["yv"])  # [NV, D, S] fp16
        yp = np.asarray(res[c]["yp"])  # [NP, T, KB*D] fp16
        for v in range(NV):
            y[c * PAIRS + v] = yv[v].T
        for q in range(NP):
            y[c * PAIRS + NV + q] = (
                yp[q].reshape(T, KB, D).transpose(1, 0, 2).reshape(S, D)
            )
    return y.reshape(B, H, S, D)
